# revision 1
# baseline (speedup 1.0000x reference)
"""BoxMultiHeadedAttention Trainium2 kernel (v2).

Self-contained: takes FULL inputs, shards batch 16 -> 8 cores x 2,
runs a Bass/Tile kernel per core via run_bass_kernel_spmd, gathers.

v2 layout/schedule changes vs v1 (313us -> ~182us):
- Q/K/V host-transposed + f16-cast (kills 48 PE transposes + casts);
  K/V rows host-pre-permuted (PSI) so all on-device writes contiguous.
- Geometry n-side broadcasts hoisted to phase-0 DMA broadcasts (off the
  critical path); deltas = DVE subtract vs per-partition m-columns.
- ACT table sets grouped Ln/Abs (deltas) -> Exp (scores) -> Sin: 2-3
  table loads instead of 7.  sin phase words (uu) computed on ACT as
  Copy(scale*dx+bias); DVE does only the 16-bit wraps (AND).
- Sin ACTs merged into quads ([128,2048] per op), exp into head pairs,
  pun STTs into head pairs: far fewer per-op overheads.
- Softmax 1/z on DVE (reciprocal_approx_fast) per head-pair, broadcast
  via a tiny k=2 selector matmul into PSUM -- the tail needs no ACT and
  no 128-descriptor broadcast DMAs.  exp pre-scaled by 1/256 (folded
  into the mask bias) so f16 ex/pun/pv never overflow.
- Strips + wpre in f16 (half the reassembly DMA bytes); batch-1 strips
  emitted between tail_a(0) and tail_b(0) to keep PE fed.
"""
import math

import numpy as np

H = 8
D = 512
DK = 64
NSEQ = 256
BL = 2            # batches per core
NCORES = 8
NMT = 2           # m tiles of 128 per batch
PI = math.pi
TWO_PI = 2 * math.pi
SBIG = 128.0      # positivity shift (periods) for fixpoint wrap
CJ = [100.0 * (1000.0 ** (-j / 8.0)) for j in range(8)]

# key-row permutation within a 128-row m tile: row r <-> box psi(r)
PSI = [(r % 8) * 16 + r // 8 for r in range(128)]   # box index for row r

_CACHE = {}


def _build_nc():
    import contextlib

    import concourse.bacc as bacc
    import concourse.bass as bass
    import concourse.tile as tile
    from concourse import mybir

    F32 = mybir.dt.float32
    F16 = mybir.dt.float16
    I32 = mybir.dt.int32
    AF = mybir.ActivationFunctionType
    ALU = mybir.AluOpType

    nc = bacc.Bacc("TRN2", target_bir_lowering=False, debug=False)

    xtq = nc.dram_tensor("xtq", [D, BL * NSEQ], F16, kind="ExternalInput")
    xtk = nc.dram_tensor("xtk", [D, BL * NSEQ], F16, kind="ExternalInput")
    xtv = nc.dram_tensor("xtv", [D, BL * NSEQ], F16, kind="ExternalInput")
    wq = nc.dram_tensor("wq", [D, D], F16, kind="ExternalInput")
    wk = nc.dram_tensor("wk", [D, D], F16, kind="ExternalInput")
    wv = nc.dram_tensor("wv", [D, D], F16, kind="ExternalInput")
    wo = nc.dram_tensor("wo", [D, D], F16, kind="ExternalInput")
    bqc = nc.dram_tensor("bqc", [D], F32, kind="ExternalInput")
    bkc = nc.dram_tensor("bkc", [D], F32, kind="ExternalInput")
    bvo = nc.dram_tensor("bvo", [1, 2 * D], F32, kind="ExternalInput")
    cxy = nc.dram_tensor("cxy", [BL, 2, 512], F32, kind="ExternalInput")
    w3h3 = nc.dram_tensor("w3h3", [BL, 512], F32, kind="ExternalInput")
    lwlh = nc.dram_tensor("lwlh", [BL, 512], F32, kind="ExternalInput")
    lwr = nc.dram_tensor("lwr", [BL, 2, NSEQ], F32, kind="ExternalInput")
    mcol = nc.dram_tensor("mcol", [BL, 128, 4], F32, kind="ExternalInput")
    mkb = nc.dram_tensor("mkb", [BL, NSEQ], F32, kind="ExternalInput")
    sel2 = nc.dram_tensor("sel2", [2, 33], F32, kind="ExternalInput")
    selp = nc.dram_tensor("selp", [2, 128], F16, kind="ExternalInput")
    # 4 contraction weight mats [128,128]: (comp dx/dy) x (f-half)
    wpc = nc.dram_tensor("wpc", [4, 128, 128], F16, kind="ExternalInput")
    mixh = nc.dram_tensor("mixh", [H, 33, 33], F16, kind="ExternalInput")
    fsc = nc.dram_tensor("fsc", [33, 2], F32, kind="ExternalInput")
    out = nc.dram_tensor("out", [BL, NSEQ, D], F32, kind="ExternalOutput")

    ctx = contextlib.ExitStack()
    with ctx:
        tc = ctx.enter_context(tile.TileContext(nc))
        singles = ctx.enter_context(tc.tile_pool(name="singles", bufs=1))
        bigseq = ctx.enter_context(tc.tile_pool(name="bigseq", bufs=1))
        work = ctx.enter_context(tc.tile_pool(name="work", bufs=4))
        outp = ctx.enter_context(tc.tile_pool(name="outp", bufs=2))
        dxp = ctx.enter_context(tc.tile_pool(name="dxp", bufs=1))
        geoctx = contextlib.ExitStack()
        geop = geoctx.enter_context(tc.tile_pool(name="geop", bufs=1))

        # ---------- constants / inputs ----------
        npi = singles.tile([128, 1], F32)
        nc.vector.memset(npi[:], -PI)
        wpct = singles.tile([128, 4, 128], F16)
        nc.gpsimd.dma_start(out=wpct[:], in_=wpc[:].rearrange("c r m -> r c m"))
        mixt = singles.tile([33, H, 33], F16)
        nc.gpsimd.dma_start(out=mixt[:], in_=mixh[:].rearrange("h a b -> a h b"))
        fsc1 = geop.tile([33, 1], F32, name="fsc1")
        nc.gpsimd.dma_start(out=fsc1[:], in_=fsc[:, 0:1])
        fsc2 = geop.tile([33, 1], F32, name="fsc2")
        nc.gpsimd.dma_start(out=fsc2[:], in_=fsc[:, 1:2])
        mv_cxy = {}
        stn_t = {}
        mkbc = {}
        for b in range(BL):
            t = geop.tile([128, 4], F32, tag=f"mcol{b}")
            nc.gpsimd.dma_start(out=t[:], in_=mcol[b, :, :])
            stn_t[b] = t
            for mt in range(NMT):
                t = singles.tile([128, 1], F32, tag=f"mkb{b}{mt}")
                nc.gpsimd.dma_start(out=t[:], in_=mkb[b, mt * 128:(mt + 1) * 128])
                mkbc[(b, mt)] = t
        selp2_t = singles.tile([2, 128], F16, name="selp2_t")
        nc.gpsimd.dma_start(out=selp2_t[:], in_=selp[:])
        bq_cols = singles.tile([128, 4], F32)
        nc.gpsimd.dma_start(out=bq_cols[:], in_=bqc[:].rearrange("(d p) -> p d", p=128))
        bk_cols = singles.tile([128, 4], F32)
        nc.gpsimd.dma_start(out=bk_cols[:], in_=bkc[:].rearrange("(d p) -> p d", p=128))

        def bcast_ap(dram_ap, parts):
            return bass.AP(
                tensor=dram_ap.tensor,
                offset=dram_ap.offset,
                ap=[[0, parts]] + list(dram_ap.ap),
            )

        cx_b = {}
        for b in range(BL):
            t = geop.tile([128, 512], F32, tag=f"cxb{b}")
            nc.scalar.dma_start(out=t[:], in_=bcast_ap(cxy[b, 0, :], 128))
            cx_b[b] = t
        wo_all = singles.tile([128, 4, D], F16, tag="wo_all")
        nc.scalar.dma_start(out=wo_all[:], in_=wo[:].rearrange("(k p) d -> p k d", p=128))

        bo_b = singles.tile([128, D], F32, tag="bo_b")
        nc.scalar.dma_start(out=bo_b[:], in_=bcast_ap(bvo[0, D:2 * D], 128))
        bv_b = singles.tile([128, D], F32, tag="bv_b")
        nc.scalar.dma_start(out=bv_b[:], in_=bcast_ap(bvo[0, 0:D], 128))
        w3_b = {}
        ll_b = {}
        lb_t = {}
        for b in range(BL):
            t = geop.tile([128, 512], F32, tag=f"w3b{b}")
            nc.sync.dma_start(out=t[:], in_=bcast_ap(w3h3[b, :], 128))
            w3_b[b] = t
        for b in range(BL):
            t = geop.tile([128, 512], F32, tag=f"llb{b}")
            nc.sync.dma_start(out=t[:], in_=bcast_ap(lwlh[b, :], 128))
            ll_b[b] = t
            t = geop.tile([33, NSEQ], F32, tag=f"lbt{b}")
            nc.sync.dma_start(out=t[0:16, :], in_=bcast_ap(lwr[b, 0, :], 16))
            nc.sync.dma_start(out=t[16:33, :], in_=bcast_ap(lwr[b, 1, :], 17))
            lb_t[b] = t

        # ---------- A phase: Ln (deltas, fac prep) then Exp (scores) ----------
        dxs = {}
        ffacs = {}

        a1ctx = contextlib.ExitStack()
        for b in range(BL):
            # fac integer phase (Sin applied later in B phase)
            ufac = work.tile([33, NSEQ], I32, tag="ufac", bufs=2)
            nc.vector.tensor_scalar(out=ufac[:], in0=lb_t[b][:], scalar1=fsc1[:],
                                    scalar2=fsc2[:], op0=ALU.mult, op1=ALU.add)
            ffac = bigseq.tile([33, NSEQ], I32, tag=f"ffac{b}")
            nc.vector.tensor_scalar(out=ffac[:], in0=ufac[:], scalar1=65535,
                                    scalar2=None, op0=ALU.bitwise_and)
            ffacs[b] = ffac
            w3p = w3_b[b]
            llp = ll_b[b]

            for mt in range(NMT):
                dx = dxp.tile([128, 512], F32, tag=f"dx{b}{mt}")
                t1 = work.tile([128, 512], F32, tag="t1", bufs=2)
                for ci in range(2):
                    nc.vector.tensor_scalar(
                        out=t1[:, ci * 256:(ci + 1) * 256],
                        in0=cx_b[b][:, ci * 256:(ci + 1) * 256],
                        scalar1=stn_t[b][:, mt * 2 + ci:mt * 2 + ci + 1],
                        scalar2=None, op0=ALU.subtract)
                t2 = work.tile([128, 512], F32, tag="t2", bufs=2)
                nc.scalar.activation(t2[:], t1[:], AF.Abs)
                t3 = work.tile([128, 512], F32, tag="t3", bufs=2)
                nc.vector.tensor_tensor(
                    out=t3[:], in0=t2[:], in1=w3p[:], op=ALU.max)
                t4 = work.tile([128, 512], F32, tag="t4", bufs=2)
                nc.scalar.activation(t4[:], t3[:], AF.Ln)
                nc.vector.tensor_tensor(
                    out=dx[:], in0=t4[:], in1=llp[:], op=ALU.subtract)
                dxs[(b, mt)] = dx
        # ---------- phase 1: projections (inputs pre-transposed) ----------
        ph1ctx = contextlib.ExitStack()
        xtp = ph1ctx.enter_context(tc.tile_pool(name="xtp", bufs=1))
        w1pool = ph1ctx.enter_context(tc.tile_pool(name="w1pool", bufs=1))
        ph1b = ph1ctx.enter_context(tc.tile_pool(name="ph1ps2", bufs=2, space="PSUM"))

        xt_q = xtp.tile([128, 4, D], F16, tag="xtq")
        nc.sync.dma_start(out=xt_q[:], in_=xtq[:].rearrange("(k p) n -> p k n", p=128))
        xt_k = xtp.tile([128, 4, D], F16, tag="xtk")
        nc.sync.dma_start(out=xt_k[:], in_=xtk[:].rearrange("(k p) n -> p k n", p=128))
        xt_v = xtp.tile([128, 4, D], F16, tag="xtv")
        nc.sync.dma_start(out=xt_v[:], in_=xtv[:].rearrange("(k p) n -> p k n", p=128))
        wq_all = w1pool.tile([128, 4, D], F16, tag="wqa")
        nc.scalar.dma_start(out=wq_all[:], in_=wq[:].rearrange("(k p) d -> p k d", p=128))
        wk_all = w1pool.tile([128, 4, D], F16, tag="wka")
        nc.scalar.dma_start(out=wk_all[:], in_=wk[:].rearrange("(k p) d -> p k d", p=128))
        wv_all = w1pool.tile([128, 4, D], F16, tag="wva")
        nc.scalar.dma_start(out=wv_all[:], in_=wv[:].rearrange("(k p) d -> p k d", p=128))


        qT = bigseq.tile([128, 4, D], F16, tag="qT")
        kT = bigseq.tile([128, 4, D], F16, tag="kT")
        for d in range(4):
            pq = ph1b.tile([128, D], F32, tag="pq")
            for k in range(4):
                nc.tensor.matmul(pq[:], wq_all[:, k, d * 128:(d + 1) * 128],
                                 xt_q[:, k, :], start=(k == 0), stop=(k == 3))
            nc.vector.tensor_scalar(out=qT[:, d, :], in0=pq[:],
                                    scalar1=bq_cols[:, d:d + 1], scalar2=None,
                                    op0=ALU.add)
            pk = ph1b.tile([128, D], F32, tag="pq")
            for k in range(4):
                nc.tensor.matmul(pk[:], wk_all[:, k, d * 128:(d + 1) * 128],
                                 xt_k[:, k, :], start=(k == 0), stop=(k == 3))
            nc.vector.tensor_scalar(out=kT[:, d, :], in0=pk[:],
                                    scalar1=bk_cols[:, d:d + 1], scalar2=None,
                                    op0=ALU.add)

        v1 = {}
        for b in range(BL):
            for h in range(H):
                for mt in range(NMT):
                    v1[(b, h, mt)] = bigseq.tile(
                        [128, 65], F16, tag=f"v1_{b}_{h}_{mt}",
                        name=f"v1_{b}_{h}_{mt}")
        for i in range(4):          # bn tile = (b, mt), rows PSI-permuted
            b, mt = divmod(i, 2)
            pv = ph1b.tile([128, D], F32, tag="pq")
            for k in range(4):
                nc.tensor.matmul(pv[:], xt_v[:, k, i * 128:(i + 1) * 128],
                                 wv_all[:, k, :], start=(k == 0), stop=(k == 3))
            for h in range(H):
                t = v1[(b, h, mt)]
                nc.vector.scalar_tensor_tensor(
                    out=t[:, 0:64], in0=pv[:, h * 64:(h + 1) * 64], scalar=1.0,
                    in1=bv_b[:, h * 64:(h + 1) * 64], op0=ALU.mult, op1=ALU.add)
                nc.vector.memset(t[:, 64:65], 1.0)

        ph1ctx.close()

        a1ctx.close()
        geoctx.close()

        # scores + exp
        a2ctx = contextlib.ExitStack()
        a2s = a2ctx.enter_context(tc.tile_pool(name="a2s", bufs=3, space="PSUM"))
        exs = {}
        for b in range(BL):
            for mt in range(NMT):
                for d in range(4):
                    ns = slice(b * 256, (b + 1) * 256)
                    mb = b * 256 + mt * 128
                    ex = bigseq.tile([128, 2, NSEQ], F16, tag=f"ex{b}{mt}{d}")
                    for r in range(2):
                        hs = slice(r * 64, (r + 1) * 64)
                        sc = a2s.tile([128, NSEQ], F32, tag="sc")
                        nc.tensor.matmul(sc[:], kT[hs, d, mb:mb + 128],
                                         qT[hs, d, ns], start=True, stop=True)
                        nc.scalar.activation(ex[:, r, :], sc[:], AF.Exp,
                                             bias=mkbc[(b, mt)][:])
                    exs[(b, mt, d)] = ex
        a2ctx.close()

        # ---------- B phase: Sin + strips + PV + tail ----------
        gpool = ctx.enter_context(tc.tile_pool(name="gpool", bufs=3))
        gp2 = ctx.enter_context(tc.tile_pool(name="gp2", bufs=4))
        stp = ctx.enter_context(tc.tile_pool(name="stp", bufs=2))
        wpre_p = ctx.enter_context(tc.tile_pool(name="wpre", bufs=6))
        punp = ctx.enter_context(tc.tile_pool(name="punp", bufs=8))


        # fac sin
        fac = {}
        for b in range(BL):
            bf = bigseq.tile([33, NSEQ], F16, tag=f"fac{b}")
            nc.scalar.activation(bf[:], ffacs[b][:], AF.Sin, bias=npi[0:33, :],
                                 scale=TWO_PI / 65536.0)
            nc.vector.memset(bf[32:33, :], 1.0)
            fac[b] = bf

        # mixed m-side factors, strip-column order
        mfac = {}
        mpctx = contextlib.ExitStack()
        mpp = mpctx.enter_context(tc.tile_pool(name="mpps", bufs=2, space="PSUM"))
        for b in range(BL):
            mt_s = bigseq.tile([33, 16, 128], F16, tag=f"mfac{b}")
            for h in range(H):
                mp = mpp.tile([33, NSEQ], F32, tag="mp")
                nc.tensor.matmul(mp[:], mixt[:, h, :], fac[b][:],
                                 start=True, stop=True)
                dst = bass.AP(
                    tensor=mt_s.tensor,
                    offset=mt_s[:].offset + h * 16,
                    ap=[list(mt_s[:].ap[0]),
                        [1024, 2], [128, 8], [1, 16]],
                )
                nc.vector.tensor_copy(out=dst, in_=mp[:])
            mfac[b] = mt_s
        mpctx.close()

        ph2b = ctx.enter_context(tc.tile_pool(name="stps", bufs=2, space="PSUM"))
        ph2c = ctx.enter_context(tc.tile_pool(name="pvps", bufs=2, space="PSUM"))
        ph2d = ctx.enter_context(tc.tile_pool(name="pops", bufs=1, space="PSUM"))
        ph2r = ctx.enter_context(tc.tile_pool(name="rbps", bufs=2, space="PSUM"))
        attn_un = {b: bigseq.tile([128, 4, NSEQ], F16, tag=f"aun{b}",
                                  name=f"aun{b}")
                   for b in range(BL)}

        def sin_block(b, mt):
            gt = gpool.tile([128, 16, 512], F16, tag="gt")
            dx = dxs[(b, mt)]
            uus = []
            for j in range(8):
                s1 = CJ[j] * 65536.0 / TWO_PI
                for trig in range(2):
                    uu = work.tile([128, 512], I32, tag="uu", bufs=4)
                    s2 = (trig * 0.25 + 0.5 + SBIG) * 65536.0
                    nc.scalar.activation(uu[:], dx[:], AF.Copy,
                                         bias=s2, scale=s1)
                    uus.append(uu)
            for jj in range(4):
                ffq = work.tile([128, 4, 512], I32, tag="ffq", bufs=2)
                for q in range(4):
                    nc.vector.tensor_scalar(
                        out=ffq[:, q, :], in0=uus[4 * jj + q][:],
                        scalar1=65535, scalar2=None, op0=ALU.bitwise_and)
                nc.scalar.activation(gt[:, 4 * jj:4 * jj + 4, :], ffq[:],
                                     AF.Sin, bias=npi[:],
                                     scale=TWO_PI / 65536.0)
            return gt

        def strip_block(b, mt, gt):
            strip_s = stp.tile([128, 8, NSEQ], F16, tag="strip_s")
            for s in range(8):
                gp = gp2.tile([128, 2, 512], F16, tag="gp")
                for fh in range(2):
                    nc.gpsimd.dma_start(
                        out=gp[:, fh, :],
                        in_=gt[s::8, fh * 8:(fh + 1) * 8, :])
                sp = ph2b.tile([128, NSEQ], F32, tag="sp")
                first = True
                for fh in range(2):
                    for comp in range(2):
                        nc.tensor.matmul(
                            sp[:], wpct[:, comp * 2 + fh, :],
                            gp[:, fh, comp * 256:(comp + 1) * 256],
                            start=first, stop=False)
                        first = False
                nc.tensor.matmul(
                    sp[:], mfac[b][:, mt * 8 + s, :],
                    fac[b][:], start=False, stop=True)
                nc.vector.tensor_copy(out=strip_s[:, s, :], in_=sp[:])
            for d in range(4):
                wp = wpre_p.tile([128, 2, NSEQ], F16, tag="wp")
                for r in range(2):
                    h = 2 * d + r
                    nc.sync.dma_start(
                        out=wp[:, r, :], in_=strip_s[h * 16:(h + 1) * 16, :, :])
                pu = punp.tile([128, 2, NSEQ], F16, tag="pu")
                nc.vector.scalar_tensor_tensor(
                    out=pu[:], in0=wp[:], scalar=1e-6, in1=exs[(b, mt, d)][:],
                    op0=ALU.max, op1=ALU.mult)
                pun[(b, d, mt)] = pu

        zzrs = {}

        def tail_a(b):
            # PV; per head-pair: z gather -> spread -> 1/z -> bcast -> norm
            zzrh = work.tile([2, 4, NSEQ], F16, tag="zzrh", bufs=2)
            zzrs[b] = zzrh
            for d in range(4):
                zz1 = work.tile([1, 2, NSEQ], F32, tag="zz1", bufs=4)
                for r in range(2):
                    h = 2 * d + r
                    pv = ph2c.tile([65, NSEQ], F32, tag="pvp")
                    for mt in range(NMT):
                        nc.tensor.matmul(pv[:], v1[(b, h, mt)][:],
                                         pun[(b, d, mt)][:, r, :],
                                         start=(mt == 0), stop=(mt == 1))
                    nc.vector.tensor_copy(out=zz1[0:1, r, :], in_=pv[64:65, :])
                    nc.scalar.activation(
                        attn_un[b][r * 64:(r + 1) * 64, d, :],
                        pv[0:64, :], AF.Copy, bias=0.0, scale=1.0)
                zz = work.tile([2, NSEQ], F32, tag="zz", bufs=4)
                nc.sync.dma_start(out=zz[:], in_=zz1[0:1, :, :])
                zzr = work.tile([2, NSEQ], F32, tag="zzr", bufs=4)
                nc.vector.reciprocal_approx_fast(out=zzr[:], in_=zz[:])
                nc.vector.tensor_copy(out=zzrh[:, d, :], in_=zzr[:])

        def tail_b(b):
            # broadcast 1/z per head pair, normalize, project
            for d in range(4):
                rbp = ph2r.tile([128, NSEQ], F32, tag="rbp")
                nc.tensor.matmul(rbp[:], selp2_t[:],
                                 zzrs[b][:, d, :], start=True, stop=True)
                for r in range(2):
                    nc.vector.scalar_tensor_tensor(
                        out=attn_un[b][r * 64:(r + 1) * 64, d, :],
                        in0=attn_un[b][r * 64:(r + 1) * 64, d, :], scalar=1.0,
                        in1=rbp[r * 64:(r + 1) * 64, :],
                        op0=ALU.mult, op1=ALU.mult)
            for bnt in range(NMT):
                po = ph2d.tile([128, D], F32, tag="po")
                for k in range(4):
                    nc.tensor.matmul(
                        po[:], attn_un[b][:, k, bnt * 128:(bnt + 1) * 128],
                        wo_all[:, k, :], start=(k == 0), stop=(k == 3))
                ot = outp.tile([128, D], F32, tag="ot")
                nc.vector.tensor_tensor(out=ot[:], in0=po[:], in1=bo_b[:],
                                        op=ALU.add)
                nc.sync.dma_start(
                    out=out[b, bnt * 128:(bnt + 1) * 128, :], in_=ot[:])

        pun = {}
        order = [(0, 0), (0, 1), (1, 0), (1, 1)]
        gts = {}
        gts[(0, 0)] = sin_block(0, 0)
        gts[(0, 1)] = sin_block(0, 1)
        strip_block(0, 0, gts[(0, 0)])
        gts[(1, 0)] = sin_block(1, 0)
        strip_block(0, 1, gts[(0, 1)])
        tail_a(0)
        gts[(1, 1)] = sin_block(1, 1)
        strip_block(1, 0, gts[(1, 0)])
        tail_b(0)
        strip_block(1, 1, gts[(1, 1)])
        tail_a(1)
        tail_b(1)

    return nc


def _host_prep(inputs):
    iq = np.ascontiguousarray(inputs["input_query"], dtype=np.float32)
    ik = np.ascontiguousarray(inputs["input_key"], dtype=np.float32)
    iv = np.ascontiguousarray(inputs["input_value"], dtype=np.float32)
    box = np.asarray(inputs["input_box"], dtype=np.float32)
    mask = np.asarray(inputs["mask"])
    Wq = np.asarray(inputs["Wq"], dtype=np.float32)
    bq = np.asarray(inputs["bq"], dtype=np.float32)
    Wk = np.asarray(inputs["Wk"], dtype=np.float32)
    bk = np.asarray(inputs["bk"], dtype=np.float32)
    Wv = np.asarray(inputs["Wv"], dtype=np.float32)
    bv = np.asarray(inputs["bv"], dtype=np.float32)
    Wo = np.asarray(inputs["Wo"], dtype=np.float32)
    bo = np.asarray(inputs["bo"], dtype=np.float32)
    WG_w = np.asarray(inputs["WG_w"], dtype=np.float32)
    WG_b = np.asarray(inputs["WG_b"], dtype=np.float32)

    scale = 1.0 / math.sqrt(DK)
    wq16 = (Wq * scale).astype(np.float16)
    bq_s = (bq * scale).astype(np.float32)

    x_min, y_min, x_max, y_max = [box[..., i] for i in range(4)]
    cx = (x_min + x_max) * 0.5
    cy = (y_min + y_max) * 0.5
    w = x_max - x_min + 1.0
    hh = y_max - y_min + 1.0
    lw = np.log(w)
    lh = np.log(hh)

    B = box.shape[0]
    cxy = np.ones((B, 2, 512), dtype=np.float32)                       # row1=ones
    cxy[:, 0, :] = np.concatenate([cx, cy], axis=1)
    w3h3 = np.concatenate([1e-3 * w, 1e-3 * hh], axis=1).astype(np.float32)
    lwlh = np.concatenate([lw, lh], axis=1).astype(np.float32)
    lwr = np.stack([lw, lh], axis=1).astype(np.float32)                # [B,2,256]

    # m-side values permuted by PSI within each 128-block
    perm = np.concatenate([np.array(PSI), 128 + np.array(PSI)])
    ik_p = ik[:, perm, :]
    iv_p = iv[:, perm, :]
    scl = 1.0 / math.sqrt(DK)
    cxp = cx[:, perm]
    cyp = cy[:, perm]
    mcol = np.zeros((B, 128, 4), dtype=np.float32)
    mcol[:, :, 0] = cxp[:, 0:128]
    mcol[:, :, 1] = cyp[:, 0:128]
    mcol[:, :, 2] = cxp[:, 128:256]
    mcol[:, :, 3] = cyp[:, 128:256]
    # -log(256) pre-scales exp() so f16 ex/pun/pv stay in range; the
    # softmax ratio pv/z is unchanged (both scale by 1/256).
    mkb = ((mask.astype(np.float32) - 1.0) * 1e9 - math.log(256.0))[:, perm]
    mkb = mkb.astype(np.float32)

    sel2 = np.zeros((2, 33), dtype=np.float32)
    sel2[0, 0:16] = 1.0
    sel2[1, 16:33] = 1.0
    selp = np.zeros((2, 128), dtype=np.float16)
    selp[0, 0:64] = 1.0
    selp[1, 64:128] = 1.0
    bvo = np.concatenate([bv, bo]).reshape(1, 2 * D).astype(np.float32)

    # contraction weights: wpc[comp*2+fh] [128 rows=(m'*8+f_loc), 128 cols=(h*16+m')]
    wpc = np.zeros((4, 128, 128), dtype=np.float32)
    for comp in range(2):
        for fh in range(2):
            for mp in range(16):
                for fl in range(8):
                    f = fh * 8 + fl          # f = j*2 + trig
                    j, trig = divmod(f, 2)
                    val_idx = (32 * trig) + comp * 8 + j
                    for h in range(H):
                        wpc[comp * 2 + fh, mp * 8 + fl, h * 16 + mp] = \
                            WG_w[h, val_idx]
    wpc = wpc.astype(np.float16)

    mixh = np.zeros((H, 33, 33), dtype=np.float32)
    for h in range(H):
        for i in (2, 3):
            for j in range(8):
                rb_s = (i - 2) * 16 + j * 2 + 0
                rb_c = rb_s + 1
                ws = WG_w[h, i * 8 + j]
                wc = WG_w[h, 32 + i * 8 + j]
                rc0 = (i - 2) * 16 + j * 2 + 0
                rc1 = rc0 + 1
                mixh[h, rb_c, rc0] = ws
                mixh[h, rb_s, rc0] = wc
                mixh[h, rb_c, rc1] = wc
                mixh[h, rb_s, rc1] = -ws
        mixh[h, 32, 32] = WG_b[h]
    mixh = mixh.astype(np.float16)

    fsc = np.zeros((33, 2), dtype=np.float32)
    for i in (2, 3):
        for j in range(8):
            for trig in range(2):
                r = (i - 2) * 16 + j * 2 + trig
                fsc[r, 0] = CJ[j] * 65536.0 / TWO_PI
                fsc[r, 1] = (trig * 0.25 + 0.5 + SBIG) * 65536.0
    fsc[32, 1] = (0.5 + SBIG) * 65536.0

    shared = dict(wq=wq16, wk=Wk.astype(np.float16), wv=Wv.astype(np.float16),
                  wo=Wo.astype(np.float16), bqc=bq_s, bkc=bk,
                  bvo=bvo, sel2=sel2, selp=selp,
                  wpc=wpc, mixh=mixh, fsc=fsc)
    in_maps = []
    for c in range(NCORES):
        sl = slice(c * BL, (c + 1) * BL)
        m = dict(shared)
        m.update(xtq=np.ascontiguousarray(
                     iq[sl].reshape(BL * NSEQ, D).T).astype(np.float16),
                 xtk=np.ascontiguousarray(
                     ik_p[sl].reshape(BL * NSEQ, D).T).astype(np.float16),
                 xtv=np.ascontiguousarray(
                     iv_p[sl].reshape(BL * NSEQ, D).T).astype(np.float16),
                 cxy=cxy[sl], w3h3=w3h3[sl], lwlh=lwlh[sl], lwr=lwr[sl],
                 mcol=mcol[sl], mkb=mkb[sl])
        in_maps.append(m)
    return in_maps


def kernel(**inputs):
    from concourse.bass_utils import run_bass_kernel_spmd

    if "nc" not in _CACHE:
        nc = _build_nc()
        nc.finalize()
        _CACHE["nc"] = nc
    nc = _CACHE["nc"]

    in_maps = _host_prep(inputs)
    res = run_bass_kernel_spmd(nc, in_maps, list(range(NCORES)))
    outs = [res.results[c]["out"] for c in range(NCORES)]
    return np.concatenate(outs, axis=0).astype(np.float32)


if __name__ == "__main__":
    nc = _build_nc()
    nc.finalize()
    print("build ok")

